# revision 10
# baseline (speedup 1.0000x reference)
"""LIF (leaky integrate-and-fire) forward kernel for Trainium2, 8-core SPMD.

Reference semantics (per element, scan over T):
    u = 0.5*u + x_t
    o_t = (u - 1 >= 0) ? 1.0 : 0.0
    u = u - o_t

Sharding: pure data parallel over batch B=32 -> 4 batches per core.
Per-core shard: x [4, 16, 128, 1024] f32; C=128 on the SBUF partition dim,
(b, h*w) on the free dim, one [128, 4096] tile per timestep, processed in
4 free-dim chunks of 1024 (= one batch each) to pipeline engines.

Algebra: track p_t = pre-reset membrane potential, m_t = (p_t>=1) - p_t
(= spike minus potential = -(post-reset potential)). Then per step:

    A (DVE stt):   m_t = (p_t is_ge 1) subtract p_t     [in0 = in1 = p_t]
    B (DVE ts 2x): h   = m_t * -0.5                     [= 0.5 * u'_t]
    DMA-CCE add:   p_{t+1} = h + x_{t+1}                [the x LOAD itself
                   accumulates into SBUF via accum_op=add on a gpsimd
                   (software-DGE) dma_start -- no separate x buffers]
    ACT:           o_t(u8) = Sign(p_t - 1)              [f32->u8 cast
                   saturates -1 -> 0 (hw-verified), giving {0,1}]

vs the baseline's 3 full-width DVE ops + ACT cast per step, the DVE now
does one 1x stt + one 2x tensor_scalar per step, the spike output is
produced directly by ACT, and the add-x runs on the DMA compute engine.
Output stores go out per chunk on the HWDGE queue (sync engine), fully
decoupled from the SWDGE accumulate-loads.

Rounding matches the f32 jax reference exactly: (p>=1) and p-o are exact,
*-0.5 is exact, and the single rounding per step is the CCE f32 add.
Only p == 1.0 exactly (sign(0)=0) can differ, which is measure-zero.

Raw bass (no TileContext); standalone wait_ge instructions (this walrus
caps embedded waits at 1 per instruction). Dependent back-to-back DVE TSP
ops are safe at these tile sizes (>=1024 free): the engine is serial and
op duration far exceeds the SBUF write-commit latency (micro-verified;
the hazard exists only for tiny tiles).
"""

import numpy as np

B, T, C, HW = 32, 16, 128, 1024
NCORES = 8
BLOC = B // NCORES   # 4 batches per core
FREE = BLOC * HW     # 4096
NCH = BLOC           # 4 chunks, one batch (1024 cols) each
CH = HW              # chunk width
NO = 2               # o (u8) slots

_cached = {}


def _build_nc():
    import concourse.bass as bass
    import concourse.mybir as mybir
    from contextlib import ExitStack

    f32 = mybir.dt.float32
    u8 = mybir.dt.uint8
    Alu = mybir.AluOpType
    Act = mybir.ActivationFunctionType

    nc = bass.Bass()
    # Const AP for the activation bias (-1.0); DMA'd in from a tiny input
    # tensor at program start (s_init gates the first activation).
    _bias = nc.alloc_sbuf_tensor("const-f32-neg1", [128, 1], f32)
    nc.const_aps.aps[(f32, -1.0)] = _bias.ap()

    x_d = nc.declare_dram_parameter("x", [BLOC, T, C, HW], f32, isOutput=False)
    b_d = nc.declare_dram_parameter("bias", [C, 1], f32, isOutput=False)
    o_d = nc.declare_dram_parameter("o", [BLOC, T, C, HW], u8, isOutput=True)

    with ExitStack() as ctx:
        p = [
            ctx.enter_context(nc.sbuf_tensor(f"p{i}", [C, FREE], f32))
            for i in range(2)
        ]
        m = ctx.enter_context(nc.sbuf_tensor("m", [C, FREE], f32))
        ot = [
            ctx.enter_context(nc.sbuf_tensor(f"ot{i}", [C, FREE], u8))
            for i in range(NO)
        ]
        # per-slot per-chunk accumulate-DMA completion
        s_p = [
            [ctx.enter_context(nc.semaphore(f"s_p{i}_{c}")) for c in range(NCH)]
            for i in range(2)
        ]
        s_out = [
            ctx.enter_context(nc.semaphore(f"s_out{i}")) for i in range(NO)
        ]
        s_dve = ctx.enter_context(nc.semaphore("s_dve"))
        s_act = ctx.enter_context(nc.semaphore("s_act"))
        s_init = ctx.enter_context(nc.semaphore("s_init"))
        block = ctx.enter_context(nc.Block())

        def chunk(ap, c):
            return ap[:, c * CH : (c + 1) * CH]

        # s_dve counts: per t in 0..T-2, ops A_t[c] (-> 8t+2c+1) and
        # B_t[c] (-> 8t+2c+2), c = 0..3.
        # s_act counts: ACT_t[c] -> 4t + c + 1.
        # s_out[slot]: out-DMA_{t,c} (slot = t%2) -> 64*(t//2) + 16*(c+1).

        @block.gpsimd
        def _(gpsimd: bass.BassEngine):
            # t = 1..15: accumulate x_t into p[t%2] (chunk c holds batch c).
            # Slot-protect (ACT_{t-2} read) is transitively covered by
            # waiting on B_{t-1}[c], which itself waits on ACT_{t-2}[c].
            for t in range(1, T):
                for c in range(NCH):
                    gpsimd.wait_ge(s_dve, 8 * (t - 1) + 2 * c + 2)
                    gpsimd.dma_start(
                        out=chunk(p[t % 2], c),
                        in_=x_d[c, t],
                        accum_op=Alu.add,
                    ).then_inc(s_p[t % 2][c], 16)

        @block.vector
        def _(vector: bass.BassEngine):
            for t in range(T - 1):
                for c in range(NCH):
                    # A_t[c]: m = (p_t >= 1) - p_t
                    vector.wait_ge(s_p[t % 2][c], 16 * (t // 2 + 1))
                    vector.scalar_tensor_tensor(
                        out=chunk(m, c), in0=chunk(p[t % 2], c), scalar=1.0,
                        in1=chunk(p[t % 2], c),
                        op0=Alu.is_ge, op1=Alu.subtract,
                    ).then_inc(s_dve, 1)
                    # B_t[c]: p[(t+1)%2] = m * -0.5   (h; x added by DMA-CCE)
                    if t >= 1:
                        # ACT_{t-1}[c] still reads p[(t+1)%2][c] (= p_{t-1})
                        vector.wait_ge(s_act, 4 * (t - 1) + c + 1)
                    vector.tensor_scalar(
                        out=chunk(p[(t + 1) % 2], c), in0=chunk(m, c),
                        scalar1=-0.5, scalar2=None, op0=Alu.mult,
                    ).then_inc(s_dve, 1)

        @block.scalar
        def _(scalar: bass.BassEngine):
            scalar.wait_ge(s_init, 16)
            for t in range(T):
                for c in range(NCH):
                    scalar.wait_ge(s_p[t % 2][c], 16 * (t // 2 + 1))
                    if t >= NO:
                        # ot[t%2][c] free once out-DMA_{t-2,c} completed
                        scalar.wait_ge(
                            s_out[t % NO],
                            64 * ((t - 2) // 2) + 16 * (c + 1),
                        )
                    scalar.activation(
                        out=chunk(ot[t % NO], c), in_=chunk(p[t % 2], c),
                        func=Act.Sign, bias=-1.0, scale=1.0,
                    ).then_inc(s_act, 1)

        @block.sync
        def _(sync: bass.BassEngine):
            # t = 0: plain load x_0 -> p[0] on the HWDGE queue (runs while
            # the SWDGE side is still setting up), plus the bias const.
            sync.dma_start(out=_bias.ap(), in_=b_d[:, :]).then_inc(s_init, 16)
            for c in range(NCH):
                sync.dma_start(
                    out=chunk(p[0], c), in_=x_d[c, 0]
                ).then_inc(s_p[0][c], 16)
            for t in range(T):
                for c in range(NCH):
                    sync.wait_ge(s_act, 4 * t + c + 1)  # ot[t][c] written
                    sync.dma_start(
                        out=o_d[c, t],
                        in_=chunk(ot[t % NO], c),
                    ).then_inc(s_out[t % NO], 16)
            # all output stores complete before kernel end
            for i in range(NO):
                n_t = (T - 1 - i) // NO + 1
                sync.wait_ge(s_out[i], 64 * n_t)

    return nc


def _get_nc():
    if "nc" not in _cached:
        _cached["nc"] = _build_nc()
    return _cached["nc"]


def kernel(x_seq: np.ndarray) -> np.ndarray:
    from concourse.bass_utils import run_bass_kernel_spmd

    x = np.ascontiguousarray(np.asarray(x_seq, dtype=np.float32)).reshape(
        B, T, C, HW
    )
    nc = _get_nc()
    bias = np.full((C, 1), -1.0, dtype=np.float32)
    in_maps = [
        {"x": x[i * BLOC : (i + 1) * BLOC], "bias": bias}
        for i in range(NCORES)
    ]
    out = run_bass_kernel_spmd(nc, in_maps, list(range(NCORES)))
    _cached["last"] = out
    res = out.results
    o = np.concatenate([r["o"] for r in res], axis=0)
    return o.reshape(B, T, C, 32, 32).astype(np.float32)
